# revision 1
# baseline (speedup 1.0000x reference)
"""CTC greedy decode kernel for Trainium2 (Bass/Tile), 8-core data-parallel.

Problem: log_probs [32, 4096, 1025] f32, input_lengths [32] i64 ->
  preds    [32, 4096] int32  (per-frame argmax)
  keep     [32, 4096] bool   (non-blank & != prev & t < len)
  max_logp [32, 4096] f32    (value at argmax)

Sharding: batch dim across 8 cores (4 utterances each). Per core:
16384 frames x 1025 vocab. Frames ride the SBUF partition dim (128
frames/tile, 128 tiles); vocab rides the free dim. Per tile the DVE
computes reduce_max (value) then max_index (argmax, first occurrence =
jnp.argmax tie-break). The CTC collapse mask is a handful of small
elementwise ops on the [128, 128] per-core result grid.
"""

from contextlib import nullcontext

import numpy as np

import concourse.bacc as bacc
import concourse.mybir as mybir
from concourse.tile import TileContext
from concourse.bass_utils import run_bass_kernel_spmd

B, T, V = 32, 4096, 1025
BLANK = 1024
NCORES = 8
BLOC = B // NCORES        # utterances per core
F = BLOC * T              # frames per core
P = 128                   # partitions
NT = F // P               # tiles per core (128)
CPU = T // P              # columns per utterance (32)
G = 2                     # tiles per DMA batch / batched reduce
NB = NT // G

_CACHE = {}


def _build_program(repeat=1, mode="mi4b", g=G, bufs=8):
    if mode in ("mi4", "mi4b"):
        g = 4
    if mode == "mi8b":
        g, bufs = 8, 3
    nc = bacc.Bacc(None, target_bir_lowering=False)
    f32 = mybir.dt.float32
    lp = nc.dram_tensor("lp", [F, V], f32, kind="ExternalInput")
    tv = nc.dram_tensor("tv", [P, NT], f32, kind="ExternalInput")
    ln = nc.dram_tensor("ln", [P, NT], f32, kind="ExternalInput")
    off = nc.dram_tensor("off", [P, NT], f32, kind="ExternalInput")
    preds_o = nc.dram_tensor("preds", [P, NT], mybir.dt.int32, kind="ExternalOutput")
    keep_o = nc.dram_tensor("keep", [P, NT], mybir.dt.int32, kind="ExternalOutput")
    mlp_o = nc.dram_tensor("maxlp", [P, NT], f32, kind="ExternalOutput")

    # frame f = n*128 + p  ->  [p, n, v]
    lp_r = lp.rearrange("(n p) v -> p n v", p=P)

    with TileContext(nc) as tc:
        with (
            tc.tile_pool(name="loads", bufs=bufs) as loads,
            tc.tile_pool(name="persist", bufs=1) as pp,
        ):
            NGRP = NT // 4
            NBIG = NT // 8
            nstage = {"mi4": NGRP, "mi4b": NGRP, "mi8b": NGRP}.get(mode, NT)
            stage = pp.tile([P, 8 * nstage], mybir.dt.uint32)
            stage3 = stage.rearrange("p (r c) -> p r c", c=nstage)
            gmax = pp.tile([P, NT], f32)
            offt = pp.tile([P, NT], f32)
            tvt = pp.tile([P, NT], f32)
            lnt = pp.tile([P, NT], f32)
            preds_f = pp.tile([P, NT], f32)
            prev_f = pp.tile([P, NT], f32)
            preds_i = pp.tile([P, NT], mybir.dt.int32)
            k1 = pp.tile([P, NT], f32)
            k2 = pp.tile([P, NT], f32)
            keep_i = pp.tile([P, NT], mybir.dt.int32)

            nc.sync.dma_start(out=tvt[:], in_=tv[:])
            nc.sync.dma_start(out=lnt[:], in_=ln[:])
            nc.sync.dma_start(out=offt[:], in_=off[:])
            # valid mask is loop-independent: compute it up front
            nc.vector.tensor_tensor(
                out=k2[:], in0=tvt[:], in1=lnt[:], op=mybir.AluOpType.is_lt
            )
            if mode == "nored":
                nc.vector.memset(gmax[:], 0.0)
            if mode == "nomi":
                nc.vector.memset(stage[:], 0)

            nb = NT // g
            loop_cm = tc.For_i(0, repeat, 1) if repeat > 1 else nullcontext()
            with loop_cm:
                if mode == "mi8b":
                    # 8-tile big: one batched reduce + two 4-tile max_index
                    # scans. Needles for both scans = the big's 8 tile-maxes;
                    # the out-of-scan half of the needle window is ignored, so
                    # slot = tile % 8 uniformly. Exactness requires no tile's
                    # max to occur bit-exactly in an earlier tile of its
                    # 4-tile scan (verified zero such collisions).
                    for m in range(NBIG):
                        i0 = m * 8
                        big = loads.tile([P, 8, V], f32, tag="big")
                        nc.sync.dma_start(
                            out=big[:, 0:4, :], in_=lp_r[:, i0 : i0 + 4, :]
                        )
                        nc.sync.dma_start(
                            out=big[:, 4:8, :], in_=lp_r[:, i0 + 4 : i0 + 8, :]
                        )
                        nc.vector.tensor_reduce(
                            out=gmax[:, i0 : i0 + 8],
                            in_=big[:],
                            axis=mybir.AxisListType.X,
                            op=mybir.AluOpType.max,
                        )
                        needles = gmax[:, i0 : i0 + 8]
                        flat = big.rearrange("p g v -> p (g v)")
                        # scan s of big m -> stage column 2m+s; useful slots
                        # are 0:4 for s=0, 4:8 for s=1 (slot = tile%8)
                        nc.vector.max_index(
                            out=stage3[:, :, 2 * m],
                            in_max=needles,
                            in_values=flat[:, 0 : 4 * V],
                        )
                        nc.vector.max_index(
                            out=stage3[:, :, 2 * m + 1],
                            in_max=needles,
                            in_values=flat[:, 4 * V : 8 * V],
                        )
                if mode == "mi4":
                    # One max_index scan per 4-tile group (8200-cycle scan,
                    # 8 needles = this group's 4 maxes + next group's 4;
                    # slots for the next group are ignored). Needle window
                    # [i0:i0+8] for groups 0..30, [NT-8:NT] for the last.
                    prev_big = None
                    for grp in range(NGRP):
                        i0 = grp * 4
                        big = loads.tile([P, 4, V], f32, tag="big")
                        nc.sync.dma_start(out=big[:], in_=lp_r[:, i0 : i0 + 4, :])
                        nc.vector.tensor_reduce(
                            out=gmax[:, i0 : i0 + 4],
                            in_=big[:],
                            axis=mybir.AxisListType.X,
                            op=mybir.AluOpType.max,
                        )
                        if grp >= 1:
                            w0 = (grp - 1) * 4
                            nc.vector.max_index(
                                out=stage3[:, :, grp - 1],
                                in_max=gmax[:, w0 : w0 + 8],
                                in_values=prev_big.rearrange("p g v -> p (g v)"),
                            )
                        prev_big = big
                    nc.vector.max_index(
                        out=stage3[:, :, NGRP - 1],
                        in_max=gmax[:, NT - 8 : NT],
                        in_values=prev_big.rearrange("p g v -> p (g v)"),
                    )
                if mode == "mi4b":
                    # Backward needle window [i0-4:i0+4]: MI(grp) depends only
                    # on reduces already emitted (in-scan needles in slots
                    # 4:8); group 0 uses [0:8] with in-scan slots 0:4.
                    # Group 0 is sub-tiled (per-tile DMA + reduce) so the DVE
                    # starts after the first 525KB instead of the full 2.1MB.
                    for grp in range(NGRP):
                        i0 = grp * 4
                        big = loads.tile([P, 4, V], f32, tag="big")
                        if grp == 0:
                            for k in range(4):
                                nc.sync.dma_start(
                                    out=big[:, k, :], in_=lp_r[:, i0 + k, :]
                                )
                                nc.vector.tensor_reduce(
                                    out=gmax[:, i0 + k : i0 + k + 1],
                                    in_=big[:, k, :],
                                    axis=mybir.AxisListType.X,
                                    op=mybir.AluOpType.max,
                                )
                        else:
                            nc.sync.dma_start(
                                out=big[:], in_=lp_r[:, i0 : i0 + 4, :]
                            )
                            nc.vector.tensor_reduce(
                                out=gmax[:, i0 : i0 + 4],
                                in_=big[:],
                                axis=mybir.AxisListType.X,
                                op=mybir.AluOpType.max,
                            )
                        w0 = 0 if grp == 0 else i0 - 4
                        nc.vector.max_index(
                            out=stage3[:, :, grp],
                            in_max=gmax[:, w0 : w0 + 8],
                            in_values=big.rearrange("p g v -> p (g v)"),
                        )
                for blk in range(0 if mode in ("mi4", "mi4b", "mi8b") else nb):
                    i0 = blk * g
                    big = loads.tile([P, g, V], f32, tag="big")
                    nc.sync.dma_start(out=big[:], in_=lp_r[:, i0 : i0 + g, :])
                    if mode == "evenred":
                        # even-width reduce (possible 2x perf mode) + fixup
                        nc.vector.tensor_reduce(
                            out=gmax[:, i0 : i0 + g],
                            in_=big[:, :, 0 : V - 1],
                            axis=mybir.AxisListType.X,
                            op=mybir.AluOpType.max,
                        )
                        nc.vector.tensor_tensor(
                            out=gmax[:, i0 : i0 + g],
                            in0=gmax[:, i0 : i0 + g],
                            in1=big[:, :, V - 1],
                            op=mybir.AluOpType.max,
                        )
                    elif mode == "poolred":
                        for k in range(g):
                            nc.vector.pool_max(
                                out=gmax[:, i0 + k : i0 + k + 1],
                                in_=big[:, k, :],
                            )
                    elif mode == "nored":
                        pass  # timing-only: skip max pass (wrong results)
                    else:
                        nc.vector.tensor_reduce(
                            out=gmax[:, i0 : i0 + g],
                            in_=big[:],
                            axis=mybir.AxisListType.X,
                            op=mybir.AluOpType.max,
                        )
                    if mode != "nomi":
                        for k in range(g):
                            i = i0 + k
                            nc.vector.max_index(
                                out=stage3[:, :, i],
                                in_max=gmax[:, i : i + 1].to_broadcast([P, 8]),
                                in_values=big[:, k, :],
                            )

            if mode in ("mi4", "mi4b", "mi8b"):
                # extract per-tile absolute indices from stage[p, slot, grp],
                # then subtract (tile%4)*V to localize within the tile.
                sel = stage.rearrange("p (r c) -> p c r", c=NGRP)  # [p, grp, slot]
                pf3 = preds_f.rearrange("p (g k) -> p g k", k=4)
                if mode == "mi4":
                    # slot k for groups 0..30, slot 4+k for the last group
                    nc.vector.tensor_copy(
                        out=pf3[:, 0 : NGRP - 1, :], in_=sel[:, 0 : NGRP - 1, 0:4]
                    )
                    nc.vector.tensor_copy(
                        out=pf3[:, NGRP - 1, :], in_=sel[:, NGRP - 1, 4:8]
                    )
                elif mode == "mi4b":
                    # slot k for group 0, slot 4+k for groups 1..31
                    nc.vector.tensor_copy(out=pf3[:, 0, :], in_=sel[:, 0, 0:4])
                    nc.vector.tensor_copy(
                        out=pf3[:, 1:NGRP, :], in_=sel[:, 1:NGRP, 4:8]
                    )
                else:
                    # slot = tile%8: even groups use slots 0:4, odd 4:8
                    sel4 = stage.rearrange("p (r c d) -> p c d r", r=8, d=2)
                    pf4 = preds_f.rearrange("p (g d k) -> p g d k", g=NBIG, d=2, k=4)
                    nc.vector.tensor_copy(
                        out=pf4[:, :, 0, :], in_=sel4[:, :, 0, 0:4]
                    )
                    nc.vector.tensor_copy(
                        out=pf4[:, :, 1, :], in_=sel4[:, :, 1, 4:8]
                    )
                nc.vector.tensor_tensor(
                    out=preds_f[:], in0=preds_f[:], in1=offt[:],
                    op=mybir.AluOpType.subtract,
                )
                nc.vector.tensor_copy(out=preds_i[:], in_=preds_f[:])
            else:
                # preds: rank-0 plane of stage, contiguous [P, NT] uint32
                top = stage[:, 0:NT]
                nc.vector.tensor_copy(out=preds_i[:], in_=top)
                nc.vector.tensor_copy(out=preds_f[:], in_=top)

            # prev (partition-shifted preds) via small SBUF->SBUF DMAs
            nc.sync.dma_start(out=prev_f[1:P, :], in_=preds_f[0 : P - 1, :])
            nc.sync.dma_start(out=prev_f[0:1, 1:NT], in_=preds_f[P - 1 : P, 0 : NT - 1])
            # sentinel -1 at utterance starts (cols 0, 32, 64, 96)
            sent = prev_f.rearrange("p (u c) -> p u c", c=CPU)[0:1, :, 0:1]
            nc.vector.memset(sent, -1.0)

            # blank-compare folds into k2 while the prev DMAs are in
            # flight; only ne(prev) + one mult remain on the serial tail
            nc.vector.tensor_scalar(
                out=k1[:], in0=preds_f[:], scalar1=float(BLANK), scalar2=None,
                op0=mybir.AluOpType.not_equal,
            )
            nc.vector.tensor_tensor(
                out=k2[:], in0=k1[:], in1=k2[:], op=mybir.AluOpType.mult
            )
            nc.vector.tensor_tensor(
                out=k1[:], in0=preds_f[:], in1=prev_f[:], op=mybir.AluOpType.not_equal
            )
            nc.vector.tensor_tensor(
                out=keep_i[:], in0=k1[:], in1=k2[:], op=mybir.AluOpType.mult
            )

            nc.sync.dma_start(out=preds_o[:], in_=preds_i[:])
            nc.sync.dma_start(out=keep_o[:], in_=keep_i[:])
            nc.sync.dma_start(out=mlp_o[:], in_=gmax[:])
    nc.compile()
    return nc


def _host_inputs(log_probs, input_lengths):
    log_probs = np.ascontiguousarray(np.asarray(log_probs, dtype=np.float32))
    lens = np.asarray(input_lengths).astype(np.int64)
    # tv[p, col] = within-utterance frame index of (p, col)
    cols = np.arange(NT)
    tvals = ((cols % CPU)[None, :] * P + np.arange(P)[:, None]).astype(np.float32)
    # off[p, col] = (col%4)*V: scan offset of tile col within its 4-tile group
    offs = np.broadcast_to(((cols % 4) * V).astype(np.float32)[None, :], (P, NT))
    offs = np.ascontiguousarray(offs, dtype=np.float32)
    in_maps = []
    for c in range(NCORES):
        lp_c = log_probs[c * BLOC : (c + 1) * BLOC].reshape(F, V)
        ln_c = lens[c * BLOC : (c + 1) * BLOC].astype(np.float32)
        ln_exp = np.broadcast_to(ln_c[cols // CPU][None, :], (P, NT))
        in_maps.append(
            {
                "lp": lp_c,
                "tv": tvals,
                "ln": np.ascontiguousarray(ln_exp, dtype=np.float32),
                "off": offs,
            }
        )
    return in_maps


def _grid_to_bt(arr):
    # arr [P, NT]: value for frame t=(col%32)*128+p of utterance col//32
    return arr.reshape(P, BLOC, CPU).transpose(1, 2, 0).reshape(BLOC, T)


def kernel(log_probs, input_lengths, **_kw):
    if "nc" not in _CACHE:
        _CACHE["nc"] = _build_program()
    nc = _CACHE["nc"]
    in_maps = _host_inputs(log_probs, input_lengths)
    res = run_bass_kernel_spmd(nc, in_maps, core_ids=list(range(NCORES)))
    preds = np.empty((B, T), dtype=np.int32)
    keep = np.empty((B, T), dtype=bool)
    max_logp = np.empty((B, T), dtype=np.float32)
    for c, r in enumerate(res.results):
        sl = slice(c * BLOC, (c + 1) * BLOC)
        preds[sl] = _grid_to_bt(r["preds"])
        keep[sl] = _grid_to_bt(r["keep"]).astype(bool)
        max_logp[sl] = _grid_to_bt(r["maxlp"])
    return preds, keep, max_logp



# revision 6
# speedup vs baseline: 1.4552x; 1.4552x over previous
"""CTC greedy decode kernel for Trainium2 (Bass/Tile), 8-core data-parallel.

Problem: log_probs [32, 4096, 1025] f32, input_lengths [32] i64 ->
  preds    [32, 4096] int32  (per-frame argmax)
  keep     [32, 4096] bool   (non-blank & != prev & t < len)
  max_logp [32, 4096] f32    (value at argmax)

Sharding: batch dim across 8 cores (4 utterances each). Per core:
16384 frames x 1025 vocab. Frames ride the SBUF partition dim (128
frames/tile, 128 tiles); vocab rides the free dim.

Argmax without a second DVE scan (the two-pass reduce+max_index version
is vector-bound at ~273us; DMA of the 67MB/core input is ~187us):

  1. DVE tensor_tensor_scan (op0=op1=max, data1=data0) computes the
     running prefix-max P_v of each frame in ONE pass. Its last element
     is the frame max m (exact f32, also the max_logp output).
  2. The Activation engine computes Sign(-P_v + m) -- 1 where P_v < m,
     0 where P_v == m -- and its accum_out sums the pass: the count of
     prefix positions strictly below the max IS the argmax index, with
     exact first-occurrence tie-breaking (jnp.argmax semantics) for any
     input, duplicates included.

So DVE does one 1.04ns/elem pass (~150us), ACT one 0.83ns/elem pass
(~137us), and the ~187us HBM stream is the critical path. The CTC
collapse mask is a handful of small [128,128] grid ops; grid columns
0:96 are finalized and stored while the tail tiles stream, so the
post-DMA tail is one tile's scan+sign plus the last prev-shift/store.
"""

import numpy as np

import concourse.bacc as bacc
import concourse.mybir as mybir
from concourse.tile import TileContext
from concourse.bass_utils import run_bass_kernel_spmd

B, T, V = 32, 4096, 1025
BLANK = 1024
NCORES = 8
BLOC = B // NCORES        # utterances per core
F = BLOC * T              # frames per core
P = 128                   # partitions
NT = F // P               # tiles per core (128)
CPU = T // P              # columns per utterance (32)
NGRP = NT // 4            # 4-tile groups
NFULL = NGRP - 1          # full groups; last 4 tiles load per-tile
CSPLIT = 96               # grid column where early/late epilogue splits

_CACHE = {}


def _build_program():
    nc = bacc.Bacc(None, target_bir_lowering=False)
    f32 = mybir.dt.float32
    i32 = mybir.dt.int32
    lp = nc.dram_tensor("lp", [F, V], f32, kind="ExternalInput")
    valid = nc.dram_tensor("valid", [P, NT], f32, kind="ExternalInput")
    preds_o = nc.dram_tensor("preds", [P, NT], i32, kind="ExternalOutput")
    keep_o = nc.dram_tensor("keep", [P, NT], i32, kind="ExternalOutput")
    mlp_o = nc.dram_tensor("maxlp", [P, NT], f32, kind="ExternalOutput")

    # frame f = n*128 + p  ->  [p, n, v]
    lp_r = lp.rearrange("(n p) v -> p n v", p=P)
    SIGN = mybir.ActivationFunctionType.Sign

    with TileContext(nc) as tc:
        with (
            tc.tile_pool(name="loads", bufs=5) as loads,
            tc.tile_pool(name="pms", bufs=3) as pms,
            tc.tile_pool(name="sgs", bufs=2) as sgs,
            tc.tile_pool(name="persist", bufs=1) as pp,
        ):
            first = loads.tile([P, 4, V], f32, tag="big")
            nc.sync.dma_start(out=first[:], in_=lp_r[:, 0:4, :])

            idxf = pp.tile([P, NT], f32)     # argmax index (exact int in f32)
            gmax = pp.tile([P, NT], f32)     # frame max (max_logp output)
            prev = pp.tile([P, NT], f32)
            validt = pp.tile([P, NT], f32)
            k1 = pp.tile([P, NT], f32)
            kp = pp.tile([P, NT], f32)
            preds_i = pp.tile([P, NT], i32)
            keep_i = pp.tile([P, NT], i32)
            pmtail = pp.tile([P, 4, V], f32)

            nc.sync.dma_start(out=validt[:], in_=valid[:])

            def tile_pass(src2d, pm2d, col):
                # one frame-tile: prefix-max scan, then Sign+accumulate
                nc.vector.tensor_tensor_scan(
                    out=pm2d, data0=src2d, data1=src2d,
                    initial=-3.0e38,
                    op0=mybir.AluOpType.max, op1=mybir.AluOpType.max,
                )
                sg = sgs.tile([P, V], f32, tag="sg")
                nc.scalar.activation(
                    out=sg[:], in_=pm2d, func=SIGN,
                    bias=pm2d[:, V - 1 : V], scale=-1.0,
                    accum_out=idxf[:, col : col + 1],
                )

            def finalize(c0, c1):
                # grid epilogue for columns [c0, c1): prev-shift, CTC mask,
                # int convert, store. Wrap row reads column c0-1 from idxf
                # when c0 > 0 (that column is final before this runs).
                # Emitted only after every load is queued on SP, so its
                # sem-waits never head-of-line-block the load stream.
                nc.vector.tensor_copy(
                    out=preds_i[:, c0:c1], in_=idxf[:, c0:c1]
                )
                nc.sync.dma_start(out=preds_o[:, c0:c1], in_=preds_i[:, c0:c1])
                nc.sync.dma_start(out=mlp_o[:, c0:c1], in_=gmax[:, c0:c1])
                nc.sync.dma_start(
                    out=prev[1:P, c0:c1], in_=idxf[0 : P - 1, c0:c1]
                )
                w0 = max(c0, 1)
                nc.sync.dma_start(
                    out=prev[0:1, w0:c1], in_=idxf[P - 1 : P, w0 - 1 : c1 - 1]
                )
                sent = prev.rearrange("p (u c) -> p u c", c=CPU)
                for u in range(BLOC):
                    if c0 <= u * CPU < c1:
                        nc.vector.memset(sent[0:1, u : u + 1, 0:1], -1.0)
                nc.vector.tensor_scalar(
                    out=k1[:, c0:c1], in0=idxf[:, c0:c1],
                    scalar1=float(BLANK), scalar2=None,
                    op0=mybir.AluOpType.not_equal,
                )
                nc.vector.tensor_tensor(
                    out=k1[:, c0:c1], in0=k1[:, c0:c1], in1=validt[:, c0:c1],
                    op=mybir.AluOpType.mult,
                )
                nc.vector.tensor_tensor(
                    out=kp[:, c0:c1], in0=idxf[:, c0:c1], in1=prev[:, c0:c1],
                    op=mybir.AluOpType.not_equal,
                )
                nc.vector.tensor_tensor(
                    out=keep_i[:, c0:c1], in0=kp[:, c0:c1], in1=k1[:, c0:c1],
                    op=mybir.AluOpType.mult,
                )
                nc.sync.dma_start(out=keep_o[:, c0:c1], in_=keep_i[:, c0:c1])

            for g in range(NFULL):
                i0 = g * 4
                if g == 0:
                    big = first
                else:
                    big = loads.tile([P, 4, V], f32, tag="big")
                    nc.sync.dma_start(out=big[:], in_=lp_r[:, i0 : i0 + 4, :])
                pm = pms.tile([P, 4, V], f32, tag="pm")
                for i in range(4):
                    tile_pass(big[:, i, :], pm[:, i, :], i0 + i)
                nc.vector.tensor_copy(
                    out=gmax[:, i0 : i0 + 4], in_=pm[:, :, V - 1]
                )

            for k, t in enumerate(range(NFULL * 4, NT)):
                bt = loads.tile([P, 1, V], f32, tag="tail")
                nc.sync.dma_start(out=bt[:], in_=lp_r[:, t : t + 1, :])
                tile_pass(bt[:, 0, :], pmtail[:, k, :], t)
                nc.vector.tensor_copy(
                    out=gmax[:, t : t + 1], in_=pmtail[:, k, V - 1 : V]
                )
            finalize(0, CSPLIT)
            finalize(CSPLIT, NT)
    nc.compile()
    return nc


def _host_inputs(log_probs, input_lengths):
    log_probs = np.ascontiguousarray(np.asarray(log_probs, dtype=np.float32))
    lens = np.asarray(input_lengths).astype(np.int64)
    cols = np.arange(NT)
    # valid mask: frame t = (c%32)*128 + p < len(utterance c//32)
    tvals = (cols % CPU)[None, :] * P + np.arange(P)[:, None]
    in_maps = []
    for c in range(NCORES):
        lp_c = log_probs[c * BLOC : (c + 1) * BLOC].reshape(F, V)
        lens_c = lens[c * BLOC : (c + 1) * BLOC]
        vmask = (tvals < lens_c[cols // CPU][None, :]).astype(np.float32)
        in_maps.append({"lp": lp_c, "valid": np.ascontiguousarray(vmask)})
    return in_maps


def _grid_to_bt(arr):
    # arr [P, NT]: value for frame t=(col%32)*128+p of utterance col//32
    return arr.reshape(P, BLOC, CPU).transpose(1, 2, 0).reshape(BLOC, T)


def kernel(log_probs, input_lengths, **_kw):
    if "nc" not in _CACHE:
        _CACHE["nc"] = _build_program()
    nc = _CACHE["nc"]
    in_maps = _host_inputs(log_probs, input_lengths)
    res = run_bass_kernel_spmd(nc, in_maps, core_ids=list(range(NCORES)))
    preds = np.empty((B, T), dtype=np.int32)
    keep = np.empty((B, T), dtype=bool)
    max_logp = np.empty((B, T), dtype=np.float32)
    for c, r in enumerate(res.results):
        sl = slice(c * BLOC, (c + 1) * BLOC)
        preds[sl] = _grid_to_bt(r["preds"])
        keep[sl] = _grid_to_bt(r["keep"]).astype(bool)
        max_logp[sl] = _grid_to_bt(r["maxlp"])
    return preds, keep, max_logp
